# revision 1
# baseline (speedup 1.0000x reference)
"""Trainium2 Bass kernel for nn_ATNLPmodel (retrieval_knn).

Strategy (per sharding hint): shard the 16384-row snapshot database across the
8 NeuronCores (2048 rows each); queries (32 snapshots x 5 shift variants = 160
bit-vector columns, bit-channel 15 always zero -> 15 of 16 contraction chunks)
are replicated. Each core computes its local shift-maxed similarity matrix
(2048, 32) with tensor-engine matmuls (stationary = transposed db row-tile,
moving = query bit columns, fp32 PSUM accumulation over the contraction);
a DVE tensor_reduce takes the max over the 5 shift columns per query. The
host gathers the 8 local matrices and finishes the tiny global reductions.

Precision (DTYPE):
  bf16   - db rows rounded to bf16; query bits are exact 0/1. Worst-case
           rel err ~6e-5 on avg_sim; unit_sim/top_cls made exact by host
           refinement: rows within REFINE_EPS of each query's device max are
           rescored in fp32 (top1-top2 gaps are ~100x the bf16 noise).
  fp8    - e4m3 with power-of-2 prescale (db*64, bits/64 - both exact
           transforms); ~1.2e-3 rel err on avg_sim, ~25% faster.
  bf16x2 - exact-split db = hi + lo (both bf16); bits.hi + bits.lo in fp32
           PSUM reproduces fp32 to ~1e-6 at 2x the matmul/DMA cost.

Perf notes (measured via NTFF profiles on trn2; ~34.5us max / ~34.2us mean
across cores, down from a 44us baseline):
  - matmuls run at the N=160 stream roofline (~69ns/MM warm, LDWEIGHTS
    fully overlapped): 240 real MMs = 16.6us of PE stream; the rest of the
    span is the fixed wrapper preamble (~6us, excluded from exec_time),
    the DMA ramp, and a fixed ~10.9us tail (last reduce/out + kernel
    barriers + the wrapper's unconditional 249-semaphore teardown sweep).
  - HAM clock gate: the PE runs at 1.2 GHz until a free-running ~3.4us
    activity window is fully busy (flip lag measured 3.1-6.8us), and ANY
    >~1us PE idle inside a window re-throttles it. N_WARM dummy matmuls on
    a zeroed tile keep the PE busy from its first instruction, sized so
    warm-up ends ~= p95 first-data arrival (~12.5us): every real MM then
    runs at 2.4 GHz with no post-warm-up idle. (Observed pitfall: 83ns/MM
    steady = P0 power-state downclock to ~2.0 GHz, machine state, not HAM.)
  - DMA: early aggregate delivery is ~0.33 GB/us shared across ALL queues
    (HWDGE sync+scalar and SWDGE); queue startup is 1.4-3us after issue
    and varies per core. >10 tracked HWDGE transfers head-of-line-block
    the ring (12 transfers measured a ~6us regression on every core).
    db is stored partition-major so multi-tile chunks are contiguous
    3.8-7.7KB per-partition lines (1920B singles are below the ~2KB DMA
    efficiency knee). One early tile (m1) rides SWDGE - exactly one: a
    second early SWDGE tile arrives ~2.5us late (serial SWDGE rate).
  - Built on bacc.Bacc + compile(): moves/splits multi-sem waits to satisfy
    the TRN2 one-embedded-wait-per-instruction constraint.

Device layout per core:
  db_hi : (128 kp, 16 m * 15 kc * 128 rr) fp8, k = kc*128+kp on partitions
  qt    : (128 kp, 15 kc * 160 col), col = query*5 + shift
  out   : (128 rr, 16 m * 32 q) f32 - shift-maxed local sims
"""

import numpy as np
import ml_dtypes

bf16 = ml_dtypes.bfloat16

# Problem constants (hardcoded per contract - kernel.py must be self-contained)
B, Lb, Lc, C, L2, R, P = 8, 128, 512, 16, 128, 4, 2
N_DB, NCORES = 16384, 8
NSH = N_DB // NCORES          # 2048 database rows per core
K = C * L2                    # 2048 contraction length
NS = 2 * P + 1                # 5 shifts
B2 = B * R                    # 32 query snapshots
NQ = B2 * NS                  # 160 query columns
MT = NSH // 128               # 16 row tiles per core
KC = K // 128                 # 16 contraction chunks
KCU = 15                      # chunks actually used: bit-channel 15 is always
                              # zero (token ids < 30522 < 2^15), contributes 0
PAD_ID = 0
EPS = 1e-8

DTYPE = "fp8"                 # "bf16x2" (hi+lo, ~1e-6), "bf16" (~6e-5), "fp8" (~1.3e-3)
USE_LO = DTYPE == "bf16x2"
# raw-sim-unit eps for host top-k refinement (device sims are quantized;
# top1-top2 raw gaps are ~0.39, so generous epsilons still give ~1-3 rows)
REFINE_EPS = 0.25 if DTYPE == "fp8" else 0.02
FP8_SCALE = 64.0              # db*64 / bits/64: both exact transforms in e4m3
N_WARM = 44                   # PE warm-up matmuls (N=128 each, ~107ns cold)

_CACHE = {}


# ----------------------------------------------------------------- host prep

def _prep_queries(bert_input_ids, bert_offsets, slidingWindowIndex):
    ids = np.asarray(bert_input_ids).astype(np.int64)        # (B, Lb)
    offs = np.asarray(bert_offsets).astype(np.int64)         # (B, Lb, 2)
    swi = int(np.asarray(slidingWindowIndex))
    pos = np.arange(Lc)[None, :, None]
    mask = (pos >= offs[..., 0][:, None, :]) & (pos < offs[..., 1][:, None, :])
    token_idx = np.argmax(mask, axis=2)
    no_cover = ~np.any(mask, axis=2)
    seq = np.take_along_axis(ids, token_idx, axis=1)
    seq = np.where(no_cover, PAD_ID, seq)                    # (B, Lc)
    bits = ((seq[..., None] >> np.arange(C)) & 1).astype(np.float32)
    enc = bits.transpose(0, 2, 1)                            # (B, C, Lc)
    stride = (Lc - L2) // max(R - 1, 1)
    starts = np.clip(swi + np.arange(R) * stride, 0, Lc - L2)
    idx = starts[:, None] + np.arange(L2)[None, :]           # (R, L2)
    snaps = enc[:, :, idx]                                   # (B, C, R, L2)
    snaps = snaps.transpose(0, 2, 1, 3).reshape(B2, C, L2)
    nbits = snaps.reshape(B2, -1).sum(axis=1)
    inv_qnorm = (1.0 / (np.sqrt(nbits) + EPS)).astype(np.float32)
    qcols = np.empty((K, NQ), dtype=np.float32)              # col = q*NS + s
    for si, s in enumerate(range(-P, P + 1)):
        rolled = np.roll(snaps, s, axis=2).reshape(B2, K)
        qcols[:, np.arange(B2) * NS + si] = rolled.T
    qsrc = qcols / FP8_SCALE if DTYPE == "fp8" else qcols
    qt = np.ascontiguousarray(
        qsrc.reshape(KC, 128, NQ)[:KCU].transpose(1, 0, 2)   # (kp, kc, col)
    ).reshape(128, KCU * NQ).astype(_np_dt())
    ch15_zero = not qcols[KCU * 128:].any()
    return qt, inv_qnorm, qcols, ch15_zero


def _np_dt():
    import ml_dtypes
    return ml_dtypes.float8_e4m3 if DTYPE == "fp8" else bf16


def _prep_db(database):
    dbf = np.ascontiguousarray(np.asarray(database, dtype=np.float32)
                               .reshape(N_DB, K))
    ss = np.einsum('nk,nk->n', dbf, dbf)
    inv = (1.0 / (np.sqrt(ss) + EPS)).astype(np.float32)
    dbn = dbf * inv[:, None]
    # (c, m, kp, kc, rr): r = c*2048 + m*128 + rr, k = kc*128 + kp
    t = dbn.reshape(NCORES, MT, 128, KC, 128)[:, :, :, :KCU].transpose(
        0, 1, 4, 3, 2)
    if DTYPE == "fp8":
        # partition-major (c, kp, m, kc, rr): an n-tile DMA chunk is then a
        # contiguous n*1920B run per partition (1920B lines are below the
        # ~2KB DMA efficiency knee; 2-4 tile chunks give 3.8-7.7KB lines)
        hi = (t * FP8_SCALE).astype(_np_dt())
        hi = np.ascontiguousarray(hi.transpose(0, 2, 1, 3, 4)).reshape(
            NCORES, 128, MT * KCU * 128)
        lo = None
    else:
        hi = np.ascontiguousarray(t, dtype=bf16)
        if USE_LO:
            lo = (t - hi.astype(np.float32)).astype(bf16)
            lo = lo.reshape(NCORES, MT, 128, KCU * 128)
        else:
            lo = None
        hi = hi.reshape(NCORES, MT, 128, KCU * 128)
    return hi, lo, dbn


# --------------------------------------------------------------- bass kernel

def _build_nc():
    from concourse import bass, bacc, mybir, tile
    from contextlib import ExitStack

    # Bacc (not plain Bass): its compile() runs move_matmul_waits_to_ldweights
    # and generate_event_semaphores, which split multi-sem waits to satisfy the
    # TRN2 one-embedded-wait-per-instruction constraint.
    kw = dict(target_bir_lowering=False, debug=False, num_devices=NCORES)
    kw.update(_CACHE.get("bass_kwargs", {}))
    nc = bacc.Bacc("TRN2", **kw)
    mm_dt = mybir.dt.float8e4 if DTYPE == "fp8" else mybir.dt.bfloat16
    db_hi_shape = ([128, MT * KCU * 128] if DTYPE == "fp8"
                   else [MT, 128, KCU * 128])
    db_hi = nc.declare_dram_parameter("db_hi", db_hi_shape,
                                      mm_dt, isOutput=False)
    if USE_LO:
        db_lo = nc.declare_dram_parameter("db_lo", [MT, 128, KCU * 128],
                                          mm_dt, isOutput=False)
    qt = nc.declare_dram_parameter("qt", [128, KCU * NQ],
                                   mm_dt, isOutput=False)
    # out[rr, m*32+q] — host reshuffles to (2048, 32)
    out = nc.declare_dram_parameter("out", [128, MT * B2],
                                    mybir.dt.float32, isOutput=True)

    with tile.TileContext(nc) as tc, ExitStack() as ctx:
        qpool = ctx.enter_context(tc.tile_pool(name="qpool", bufs=1))
        dbpool = ctx.enter_context(
            tc.tile_pool(name="dbpool", bufs=MT if USE_LO else 1))
        pspool = ctx.enter_context(tc.tile_pool(name="pspool", bufs=6, space="PSUM"))
        opool = ctx.enter_context(tc.tile_pool(name="opool", bufs=1))
        wpool = ctx.enter_context(tc.tile_pool(name="wpool", bufs=1))
        wpspool = ctx.enter_context(tc.tile_pool(name="wps", bufs=1, space="PSUM"))

        # PE warm-up: HAM clock-gates the PE to 1.2 GHz until a full free-
        # running 3.4us activity window is busy (flip lag observed 3.1-6.8us).
        # Real matmuls can't start until qt + the first db tile land (~4us
        # into the body), so without this every core pays that lag as a
        # cold-start on real work. Dummy matmuls on a zeroed tile keep the
        # PE busy from its first instruction. N=128 keeps them fine-grained:
        # the PE stream is in-order, so the first real matmul queues behind
        # at most one 107ns warm-up rather than a 427ns N=512 one.
        warm = wpool.tile([128, 128], mm_dt)
        nc.gpsimd.memset(warm[:], 0)
        wps = wpspool.tile([128, 128], mybir.dt.float32)
        for _ in range(N_WARM):
            # lhsT and rhs share the zeroed region (read-read, two ports):
            # halves the memset on the warm-up critical path
            nc.tensor.matmul(wps[:], warm[:], warm[:],
                             start=True, stop=True)

        qt_t = qpool.tile([128, KCU * NQ], mm_dt)
        # >9-10 tracked HWDGE DMAs head-of-line-block the ring (measured:
        # splitting qt/m0 into halves, 12 transfers, regressed every core
        # by ~6us), so the plan stays at 9 HWDGE + 3 SWDGE transfers.
        nc.scalar.dma_start(out=qt_t[:], in_=qt[:])
        o_sb = opool.tile([128, MT * B2], mybir.dt.float32)

        # Chunks sized so each queue's cumulative delivery stays ahead of
        # the warm PE's consumption deadline (tile m needed at ~10.5 +
        # 1.04*m us; measured queue rates ~0.15-0.17 GB/us each + ~0.6us
        # per-transfer dead time). m1 rides the otherwise-idle SWDGE
        # (gpsimd) queue so the in-order PE never bubbles at tile 1 (one
        # SWDGE input tile only: a second, m2, measured ~2.5us late on
        # every core - SWDGE's serial rate can't feed two early tiles).
        # (early aggregate delivery is a fixed ~0.33GB/us shared across ALL
        # queues: adding SWDGE input tiles or finer splits just moves the
        # bubble - measured repeatedly - so keep exactly this shape)
        chunks = [(0, 1), (1, 1), (2, 2), (4, 2), (6, 2), (8, 3), (11, 3),
                  (14, 2)]
        engines = [nc.sync, nc.gpsimd, nc.sync, nc.scalar, nc.sync,
                   nc.scalar, nc.sync, nc.scalar]
        hi_tiles = {}
        if not USE_LO:
            for (mstart, n), eng in zip(chunks, engines):
                t = dbpool.tile([128, n * KCU * 128], mm_dt,
                                tag=f"hi{mstart}")
                eng.dma_start(
                    out=t[:],
                    in_=db_hi[:, mstart * KCU * 128:(mstart + n) * KCU * 128])
                for mm in range(n):
                    hi_tiles[mstart + mm] = (t, mm)

        n_acc = 2 * KCU if USE_LO else KCU
        for m in range(MT):
            if USE_LO:
                eng = nc.sync if m % 2 == 0 else nc.scalar
                eng2 = nc.scalar if m % 2 == 0 else nc.sync
                hi_t = dbpool.tile([128, KCU * 128], mybir.dt.bfloat16, tag="hi")
                eng.dma_start(out=hi_t[:], in_=db_hi[m])
                lo_t = dbpool.tile([128, KCU * 128], mybir.dt.bfloat16, tag="lo")
                eng2.dma_start(out=lo_t[:], in_=db_lo[m])
                moff = 0
            else:
                hi_t, moff = hi_tiles[m]
            ps = pspool.tile([128, NQ], mybir.dt.float32, tag="ps")
            i = 0
            for kc in range(KCU):
                nc.tensor.matmul(
                    ps[:],
                    hi_t[:, (moff * KCU + kc) * 128:(moff * KCU + kc + 1) * 128],
                    qt_t[:, kc * NQ:(kc + 1) * NQ],
                    start=(i == 0), stop=(i == n_acc - 1))
                i += 1
                if USE_LO:
                    nc.tensor.matmul(
                        ps[:],
                        lo_t[:, kc * 128:(kc + 1) * 128],
                        qt_t[:, kc * NQ:(kc + 1) * NQ],
                        start=False, stop=(i == n_acc - 1))
                    i += 1
            nc.vector.tensor_reduce(
                o_sb[:, m * B2:(m + 1) * B2],
                ps[:].rearrange("p (q s) -> p q s", s=NS),
                axis=mybir.AxisListType.X,
                op=mybir.AluOpType.max)
            if m == MT // 2 - 1:
                # first half of the output leaves while the PE still streams
                # (on the otherwise-idle SWDGE path: its sem wait must not
                # block the HWDGE engines' input-DMA issue)
                half = MT // 2 * B2
                nc.gpsimd.dma_start(out=out[:, :half], in_=o_sb[:, :half])
            elif m == 3 * MT // 4 - 1:
                # third quarter too, so the final post-matmul DMA is small
                q3, q4 = MT // 2 * B2, 3 * MT // 4 * B2
                nc.sync.dma_start(out=out[:, q3:q4], in_=o_sb[:, q3:q4])
        q4 = 3 * MT // 4 * B2
        nc.sync.dma_start(out=out[:, q4:], in_=o_sb[:, q4:])
    nc.compile()
    return nc


def _get_nc():
    if "nc" not in _CACHE:
        _CACHE["nc"] = _build_nc()
    return _CACHE["nc"]


# ---------------------------------------------------------------- entry point

def _run_device(in_maps, trace=False, trace_cores=None):
    from concourse.bass_utils import run_bass_kernel_spmd
    return run_bass_kernel_spmd(_get_nc(), in_maps, list(range(NCORES)),
                                trace=trace, trace_cores=trace_cores)


def _get_runner():
    """Cached jitted SPMD runner (avoids per-call retrace/recompile of the
    run_bass_via_pjrt path)."""
    if "runner" in _CACHE:
        return _CACHE["runner"]
    import jax
    from jax.experimental.shard_map import shard_map
    from jax.sharding import Mesh, PartitionSpec
    from concourse import bass2jax

    nc = _get_nc()
    bass2jax.install_neuronx_cc_hook()
    in_names, out_names, out_avals, zero_outs = [], [], [], []
    from concourse import mybir
    partition_name = (nc.partition_id_tensor.name
                      if nc.partition_id_tensor else None)
    for alloc in nc.m.functions[0].allocations:
        if not isinstance(alloc, mybir.MemoryLocationSet):
            continue
        name = alloc.memorylocations[0].name
        if alloc.kind == "ExternalInput":
            if name != partition_name:
                in_names.append(name)
        elif alloc.kind == "ExternalOutput":
            out_names.append(name)
            shape = tuple(alloc.tensor_shape)
            dtype = mybir.dt.np(alloc.dtype)
            out_avals.append(jax.core.ShapedArray(shape, dtype))
            zero_outs.append(np.zeros(shape, dtype))
    n_params = len(in_names)
    all_in_names = in_names + out_names + (
        [partition_name] if partition_name else [])

    def _body(*args):
        operands = list(args)
        if partition_name is not None:
            operands.append(bass2jax.partition_id_tensor())
        outs = bass2jax._bass_exec_p.bind(
            *operands,
            out_avals=tuple(out_avals),
            in_names=tuple(all_in_names),
            out_names=tuple(out_names),
            lowering_input_output_aliases=(),
            sim_require_finite=True,
            sim_require_nnan=True,
            nc=nc,
        )
        return tuple(outs)

    devices = jax.devices()[:NCORES]
    mesh = Mesh(np.asarray(devices), ("core",))
    n_outs = len(out_names)
    sharded = jax.jit(
        shard_map(_body, mesh=mesh,
                  in_specs=(PartitionSpec("core"),) * (n_params + n_outs),
                  out_specs=(PartitionSpec("core"),) * n_outs,
                  check_rep=False),
        donate_argnums=tuple(range(n_params, n_params + n_outs)),
        keep_unused=True)

    def runner(in_maps):
        concat_in = [np.concatenate([m[name] for m in in_maps], axis=0)
                     for name in in_names]
        concat_zeros = [np.zeros((NCORES * z.shape[0], *z.shape[1:]), z.dtype)
                        for z in zero_outs]
        out_arrs = sharded(*concat_in, *concat_zeros)
        return [
            {name: np.asarray(out_arrs[i]).reshape(NCORES, *out_avals[i].shape)[c]
             for i, name in enumerate(out_names)}
            for c in range(NCORES)]

    _CACHE["runner"] = runner
    return runner


def _refine(sim, dbn, qcols, db_classes, inv_qnorm):
    """Exact fp32 rescore of rows within REFINE_EPS of each query's device max
    (device sims are bf16-rounded; top1-top2 gaps are ~100x the bf16 error)."""
    mx = sim.max(axis=0)
    unit = np.empty(B2, dtype=np.float32)
    top = np.empty(B2, dtype=np.int32)
    cls = np.asarray(db_classes).astype(np.int32)
    for q in range(B2):
        cand = np.nonzero(sim[:, q] >= mx[q] - REFINE_EPS)[0]
        exact = (dbn[cand] @ qcols[:, q * NS:(q + 1) * NS]).max(axis=1)
        best = int(np.argmax(exact))
        unit[q] = exact[best] * inv_qnorm[q]
        top[q] = cls[cand[best]]
    return unit, top


def kernel(bert_input_ids, bert_offsets, database, db_classes,
           slidingWindowIndex, _trace=False):
    qt, inv_qnorm, qcols, ch15_zero = _prep_queries(
        bert_input_ids, bert_offsets, slidingWindowIndex)
    hi, lo, dbn = _prep_db(database)
    if not ch15_zero:
        # never per input spec (ids < 30522 < 2^15); exact host fallback
        sim = (dbn @ qcols).reshape(N_DB, B2, NS).max(axis=2)
        avg_sim = (sim.mean(axis=0) * inv_qnorm).astype(np.float32)
        unit_sim = (sim.max(axis=0) * inv_qnorm).astype(np.float32)
        top_cls = np.asarray(db_classes).astype(np.int32)[np.argmax(sim, axis=0)]
        if _trace:
            return (unit_sim, top_cls, avg_sim), None
        return unit_sim, top_cls, avg_sim
    in_maps = []
    for c in range(NCORES):
        m = {"db_hi": hi[c], "qt": qt}
        if USE_LO:
            m["db_lo"] = lo[c]
        in_maps.append(m)
    if _trace:
        res = _run_device(in_maps, trace=True,
                          trace_cores=list(range(NCORES)))
        results = res.results
    else:
        res = None
        results = _get_runner()(in_maps)
    sim = np.concatenate(
        [results[c]["out"].reshape(128, MT, B2).transpose(1, 0, 2)
         .reshape(NSH, B2) for c in range(NCORES)], axis=0)
    avg_sim = (sim.mean(axis=0) * inv_qnorm).astype(np.float32)
    if USE_LO:
        unit_sim = (sim.max(axis=0) * inv_qnorm).astype(np.float32)
        top_cls = np.asarray(db_classes).astype(np.int32)[
            np.argmax(sim, axis=0)]
    else:
        unit_sim, top_cls = _refine(sim, dbn, qcols, db_classes, inv_qnorm)
    if _trace:
        return (unit_sim, top_cls, avg_sim), res
    return unit_sim, top_cls, avg_sim



# revision 3
# speedup vs baseline: 1.1060x; 1.1060x over previous
"""Trainium2 Bass kernel for nn_ATNLPmodel (retrieval_knn).

Strategy (per sharding hint): shard the 16384-row snapshot database across the
8 NeuronCores (2048 rows each); queries (32 snapshots x 5 shift variants = 160
bit-vector columns, bit-channel 15 always zero -> 15 of 16 contraction chunks)
are replicated. Each core computes its local shift-maxed similarity matrix
(2048, 32) with tensor-engine matmuls (stationary = transposed db row-tile,
moving = query bit columns, fp32 PSUM accumulation over the contraction);
a DVE tensor_reduce takes the max over the 5 shift columns per query. The
host gathers the 8 local matrices and finishes the tiny global reductions.

Precision (DTYPE):
  bf16   - db rows rounded to bf16; query bits are exact 0/1. Worst-case
           rel err ~6e-5 on avg_sim; unit_sim/top_cls made exact by host
           refinement: rows within REFINE_EPS of each query's device max are
           rescored in fp32 (top1-top2 gaps are ~100x the bf16 noise).
  fp8    - e4m3 with power-of-2 prescale (db*64, bits/64 - both exact
           transforms); ~1.2e-3 rel err on avg_sim, ~25% faster.
  bf16x2 - exact-split db = hi + lo (both bf16); bits.hi + bits.lo in fp32
           PSUM reproduces fp32 to ~1e-6 at 2x the matmul/DMA cost.

Perf notes (measured via NTFF profiles on trn2; ~34.5us max / ~34.2us mean
across cores, down from a 44us baseline):
  - matmuls run at the N=160 stream roofline (~69ns/MM warm, LDWEIGHTS
    fully overlapped): 240 real MMs = 16.6us of PE stream; the rest of the
    span is the fixed wrapper preamble (~6us, excluded from exec_time),
    the DMA ramp, and a fixed ~10.9us tail (last reduce/out + kernel
    barriers + the wrapper's unconditional 249-semaphore teardown sweep).
  - HAM clock gate: the PE runs at 1.2 GHz until a free-running ~3.4us
    activity window is fully busy (flip lag measured 3.1-6.8us), and ANY
    >~1us PE idle inside a window re-throttles it. N_WARM dummy matmuls on
    a zeroed tile keep the PE busy from its first instruction, sized so
    warm-up ends ~= p95 first-data arrival (~12.5us): every real MM then
    runs at 2.4 GHz with no post-warm-up idle. (Observed pitfall: 83ns/MM
    steady = P0 power-state downclock to ~2.0 GHz, machine state, not HAM.)
  - DMA: early aggregate delivery is ~0.33 GB/us shared across ALL queues
    (HWDGE sync+scalar and SWDGE); queue startup is 1.4-3us after issue
    and varies per core. >10 tracked HWDGE transfers head-of-line-block
    the ring (12 transfers measured a ~6us regression on every core).
    db is stored partition-major so multi-tile chunks are contiguous
    3.8-7.7KB per-partition lines (1920B singles are below the ~2KB DMA
    efficiency knee). One early tile (m1) rides SWDGE - exactly one: a
    second early SWDGE tile arrives ~2.5us late (serial SWDGE rate).
  - Built on bacc.Bacc + compile(): moves/splits multi-sem waits to satisfy
    the TRN2 one-embedded-wait-per-instruction constraint.

Device layout per core:
  db_hi : (128 kp, 16 m * 15 kc * 128 rr) fp8, k = kc*128+kp on partitions
  qt    : (128 kp, 15 kc * 160 col), col = query*5 + shift
  out   : (128 rr, 16 m * 32 q) f32 - shift-maxed local sims
"""

import numpy as np
import ml_dtypes

bf16 = ml_dtypes.bfloat16

# Problem constants (hardcoded per contract - kernel.py must be self-contained)
B, Lb, Lc, C, L2, R, P = 8, 128, 512, 16, 128, 4, 2
N_DB, NCORES = 16384, 8
NSH = N_DB // NCORES          # 2048 database rows per core
K = C * L2                    # 2048 contraction length
NS = 2 * P + 1                # 5 shifts
B2 = B * R                    # 32 query snapshots
NQ = B2 * NS                  # 160 query columns
MT = NSH // 128               # 16 row tiles per core
KC = K // 128                 # 16 contraction chunks
KCU = 15                      # chunks actually used: bit-channel 15 is always
                              # zero (token ids < 30522 < 2^15), contributes 0
PAD_ID = 0
EPS = 1e-8

DTYPE = "fp8"                 # "bf16x2" (hi+lo, ~1e-6), "bf16" (~6e-5), "fp8" (~1.3e-3)
USE_LO = DTYPE == "bf16x2"
USE_DR = DTYPE == "fp8"       # fp8 DoubleRow: one PE pass contracts 2 chunks
# raw-sim-unit eps for host top-k refinement (device sims are quantized;
# top1-top2 raw gaps are ~0.39, so generous epsilons still give ~1-3 rows)
REFINE_EPS = 0.25 if DTYPE == "fp8" else 0.02
FP8_SCALE = 64.0              # db*64 / bits/64: both exact transforms in e4m3
N_WARM = 44                   # PE warm-up matmuls (N=128 each, ~107ns cold)

_CACHE = {}


# ----------------------------------------------------------------- host prep

def _prep_queries(bert_input_ids, bert_offsets, slidingWindowIndex):
    ids = np.asarray(bert_input_ids).astype(np.int64)        # (B, Lb)
    offs = np.asarray(bert_offsets).astype(np.int64)         # (B, Lb, 2)
    swi = int(np.asarray(slidingWindowIndex))
    pos = np.arange(Lc)[None, :, None]
    mask = (pos >= offs[..., 0][:, None, :]) & (pos < offs[..., 1][:, None, :])
    token_idx = np.argmax(mask, axis=2)
    no_cover = ~np.any(mask, axis=2)
    seq = np.take_along_axis(ids, token_idx, axis=1)
    seq = np.where(no_cover, PAD_ID, seq)                    # (B, Lc)
    bits = ((seq[..., None] >> np.arange(C)) & 1).astype(np.float32)
    enc = bits.transpose(0, 2, 1)                            # (B, C, Lc)
    stride = (Lc - L2) // max(R - 1, 1)
    starts = np.clip(swi + np.arange(R) * stride, 0, Lc - L2)
    idx = starts[:, None] + np.arange(L2)[None, :]           # (R, L2)
    snaps = enc[:, :, idx]                                   # (B, C, R, L2)
    snaps = snaps.transpose(0, 2, 1, 3).reshape(B2, C, L2)
    nbits = snaps.reshape(B2, -1).sum(axis=1)
    inv_qnorm = (1.0 / (np.sqrt(nbits) + EPS)).astype(np.float32)
    qcols = np.empty((K, NQ), dtype=np.float32)              # col = q*NS + s
    for si, s in enumerate(range(-P, P + 1)):
        rolled = np.roll(snaps, s, axis=2).reshape(B2, K)
        qcols[:, np.arange(B2) * NS + si] = rolled.T
    qsrc = qcols / FP8_SCALE if DTYPE == "fp8" else qcols
    qt = np.ascontiguousarray(
        qsrc.reshape(KC, 128, NQ)[:KCU].transpose(1, 0, 2)   # (kp, kc, col)
    ).reshape(128, KCU * NQ).astype(_np_dt())
    ch15_zero = not qcols[KCU * 128:].any()
    return qt, inv_qnorm, qcols, ch15_zero


def _np_dt():
    import ml_dtypes
    return ml_dtypes.float8_e4m3 if DTYPE == "fp8" else bf16


def _prep_db(database):
    dbf = np.ascontiguousarray(np.asarray(database, dtype=np.float32)
                               .reshape(N_DB, K))
    ss = np.einsum('nk,nk->n', dbf, dbf)
    inv = (1.0 / (np.sqrt(ss) + EPS)).astype(np.float32)
    dbn = dbf * inv[:, None]
    # (c, m, kp, kc, rr): r = c*2048 + m*128 + rr, k = kc*128 + kp
    t = dbn.reshape(NCORES, MT, 128, KC, 128)[:, :, :, :KCU].transpose(
        0, 1, 4, 3, 2)
    if DTYPE == "fp8":
        # partition-major (c, kp, m, kc, rr): an n-tile DMA chunk is then a
        # contiguous n*1920B run per partition (1920B lines are below the
        # ~2KB DMA efficiency knee; 2-4 tile chunks give 3.8-7.7KB lines)
        hi = (t * FP8_SCALE).astype(_np_dt())
        hi = np.ascontiguousarray(hi.transpose(0, 2, 1, 3, 4)).reshape(
            NCORES, 128, MT * KCU * 128)
        lo = None
    else:
        hi = np.ascontiguousarray(t, dtype=bf16)
        if USE_LO:
            lo = (t - hi.astype(np.float32)).astype(bf16)
            lo = lo.reshape(NCORES, MT, 128, KCU * 128)
        else:
            lo = None
        hi = hi.reshape(NCORES, MT, 128, KCU * 128)
    return hi, lo, dbn


# --------------------------------------------------------------- bass kernel

def _build_nc():
    from concourse import bass, bacc, mybir, tile
    from contextlib import ExitStack

    # Bacc (not plain Bass): its compile() runs move_matmul_waits_to_ldweights
    # and generate_event_semaphores, which split multi-sem waits to satisfy the
    # TRN2 one-embedded-wait-per-instruction constraint.
    kw = dict(target_bir_lowering=False, debug=False, num_devices=NCORES)
    kw.update(_CACHE.get("bass_kwargs", {}))
    nc = bacc.Bacc("TRN2", **kw)
    mm_dt = mybir.dt.float8e4 if DTYPE == "fp8" else mybir.dt.bfloat16
    db_hi_shape = ([128, MT * KCU * 128] if DTYPE == "fp8"
                   else [MT, 128, KCU * 128])
    db_hi = nc.declare_dram_parameter("db_hi", db_hi_shape,
                                      mm_dt, isOutput=False)
    if USE_LO:
        db_lo = nc.declare_dram_parameter("db_lo", [MT, 128, KCU * 128],
                                          mm_dt, isOutput=False)
    qt = nc.declare_dram_parameter("qt", [128, KCU * NQ],
                                   mm_dt, isOutput=False)
    # out[rr, m*32+q] — host reshuffles to (2048, 32)
    out = nc.declare_dram_parameter("out", [128, MT * B2],
                                    mybir.dt.float32, isOutput=True)

    with tile.TileContext(nc) as tc, ExitStack() as ctx:
        qpool = ctx.enter_context(tc.tile_pool(name="qpool", bufs=1))
        dbpool = ctx.enter_context(
            tc.tile_pool(name="dbpool", bufs=MT if USE_LO else 1))
        pspool = ctx.enter_context(tc.tile_pool(name="pspool", bufs=6, space="PSUM"))
        opool = ctx.enter_context(tc.tile_pool(name="opool", bufs=1))
        wpool = ctx.enter_context(tc.tile_pool(name="wpool", bufs=1))
        wpspool = ctx.enter_context(tc.tile_pool(name="wps", bufs=1, space="PSUM"))

        # PE warm-up: HAM clock-gates the PE to 1.2 GHz until a full free-
        # running 3.4us activity window is busy (flip lag observed 3.1-6.8us).
        # Real matmuls can't start until qt + the first db tile land (~4us
        # into the body), so without this every core pays that lag as a
        # cold-start on real work. Dummy matmuls on a zeroed tile keep the
        # PE busy from its first instruction. N=128 keeps them fine-grained:
        # the PE stream is in-order, so the first real matmul queues behind
        # at most one 107ns warm-up rather than a 427ns N=512 one.
        warm = wpool.tile([128, 128], mm_dt)
        nc.gpsimd.memset(warm[:], 0)
        wps = wpspool.tile([128, 128], mybir.dt.float32)
        for _ in range(N_WARM):
            # lhsT and rhs share the zeroed region (read-read, two ports):
            # halves the memset on the warm-up critical path
            nc.tensor.matmul(wps[:], warm[:], warm[:],
                             start=True, stop=True)

        qt_t = qpool.tile([128, KCU * NQ], mm_dt)
        # >9-10 tracked HWDGE DMAs head-of-line-block the ring (measured:
        # splitting qt/m0 into halves, 12 transfers, regressed every core
        # by ~6us), so the plan stays at 9 HWDGE + 3 SWDGE transfers.
        nc.scalar.dma_start(out=qt_t[:], in_=qt[:])
        o_sb = opool.tile([128, MT * B2], mybir.dt.float32)

        # Chunks sized so each queue's cumulative delivery stays ahead of
        # the warm PE's consumption deadline (tile m needed at ~10.5 +
        # 1.04*m us; measured queue rates ~0.15-0.17 GB/us each + ~0.6us
        # per-transfer dead time). m1 rides the otherwise-idle SWDGE
        # (gpsimd) queue so the in-order PE never bubbles at tile 1 (one
        # SWDGE input tile only: a second, m2, measured ~2.5us late on
        # every core - SWDGE's serial rate can't feed two early tiles).
        # (early aggregate delivery is a fixed ~0.33GB/us shared across ALL
        # queues: adding SWDGE input tiles or finer splits just moves the
        # bubble - measured repeatedly - so keep exactly this shape)
        chunks = [(0, 1), (1, 1), (2, 2), (4, 2), (6, 2), (8, 3), (11, 3),
                  (14, 2)]
        engines = [nc.sync, nc.gpsimd, nc.sync, nc.scalar, nc.sync,
                   nc.scalar, nc.sync, nc.scalar]
        hi_tiles = {}
        if not USE_LO:
            for (mstart, n), eng in zip(chunks, engines):
                t = dbpool.tile([128, n * KCU * 128], mm_dt,
                                tag=f"hi{mstart}")
                eng.dma_start(
                    out=t[:],
                    in_=db_hi[:, mstart * KCU * 128:(mstart + n) * KCU * 128])
                for mm in range(n):
                    hi_tiles[mstart + mm] = (t, mm)

        n_acc = 2 * KCU if USE_LO else KCU
        for m in range(MT):
            if USE_LO:
                eng = nc.sync if m % 2 == 0 else nc.scalar
                eng2 = nc.scalar if m % 2 == 0 else nc.sync
                hi_t = dbpool.tile([128, KCU * 128], mybir.dt.bfloat16, tag="hi")
                eng.dma_start(out=hi_t[:], in_=db_hi[m])
                lo_t = dbpool.tile([128, KCU * 128], mybir.dt.bfloat16, tag="lo")
                eng2.dma_start(out=lo_t[:], in_=db_lo[m])
                moff = 0
            else:
                hi_t, moff = hi_tiles[m]
            ps = pspool.tile([128, NQ], mybir.dt.float32, tag="ps")
            if USE_DR:
                # fp8 DoubleRow: each PE pass contracts a PAIR of 128-deep
                # chunks (lhsT/rhs carry the pair as dim1 of a 3D AP) at
                # roughly half the stream cycles of two normal matmuls.
                # 15 chunks = 7 pairs + 1 trailing normal matmul.
                for kp in range(KCU // 2):
                    kc = 2 * kp
                    base = (moff * KCU + kc) * 128
                    nc.tensor.matmul(
                        ps[:],
                        hi_t[:, base:base + 256].rearrange(
                            "p (two r) -> p two r", two=2),
                        qt_t[:, kc * NQ:(kc + 2) * NQ].rearrange(
                            "p (two n) -> p two n", two=2),
                        start=(kp == 0), stop=False,
                        perf_mode=mybir.MatmulPerfMode.DoubleRow)
                kc = KCU - 1
                nc.tensor.matmul(
                    ps[:],
                    hi_t[:, (moff * KCU + kc) * 128:(moff * KCU + kc + 1) * 128],
                    qt_t[:, kc * NQ:(kc + 1) * NQ],
                    start=False, stop=True)
            else:
                i = 0
                for kc in range(KCU):
                    nc.tensor.matmul(
                        ps[:],
                        hi_t[:, (moff * KCU + kc) * 128:(moff * KCU + kc + 1) * 128],
                        qt_t[:, kc * NQ:(kc + 1) * NQ],
                        start=(i == 0), stop=(i == n_acc - 1))
                    i += 1
                    if USE_LO:
                        nc.tensor.matmul(
                            ps[:],
                            lo_t[:, kc * 128:(kc + 1) * 128],
                            qt_t[:, kc * NQ:(kc + 1) * NQ],
                            start=False, stop=(i == n_acc - 1))
                        i += 1
            nc.vector.tensor_reduce(
                o_sb[:, m * B2:(m + 1) * B2],
                ps[:].rearrange("p (q s) -> p q s", s=NS),
                axis=mybir.AxisListType.X,
                op=mybir.AluOpType.max)
            if m == MT // 2 - 1:
                # first half of the output leaves while the PE still streams
                # (on the otherwise-idle SWDGE path: its sem wait must not
                # block the HWDGE engines' input-DMA issue)
                half = MT // 2 * B2
                nc.gpsimd.dma_start(out=out[:, :half], in_=o_sb[:, :half])
            elif m == 3 * MT // 4 - 1:
                # third quarter too, so the final post-matmul DMA is small
                q3, q4 = MT // 2 * B2, 3 * MT // 4 * B2
                nc.sync.dma_start(out=out[:, q3:q4], in_=o_sb[:, q3:q4])
        q4 = 3 * MT // 4 * B2
        nc.sync.dma_start(out=out[:, q4:], in_=o_sb[:, q4:])
    nc.compile()
    return nc


def _get_nc():
    if "nc" not in _CACHE:
        _CACHE["nc"] = _build_nc()
    return _CACHE["nc"]


# ---------------------------------------------------------------- entry point

def _run_device(in_maps, trace=False, trace_cores=None):
    from concourse.bass_utils import run_bass_kernel_spmd
    return run_bass_kernel_spmd(_get_nc(), in_maps, list(range(NCORES)),
                                trace=trace, trace_cores=trace_cores)


def _get_runner():
    """Cached jitted SPMD runner (avoids per-call retrace/recompile of the
    run_bass_via_pjrt path)."""
    if "runner" in _CACHE:
        return _CACHE["runner"]
    import jax
    from jax.experimental.shard_map import shard_map
    from jax.sharding import Mesh, PartitionSpec
    from concourse import bass2jax

    nc = _get_nc()
    bass2jax.install_neuronx_cc_hook()
    in_names, out_names, out_avals, zero_outs = [], [], [], []
    from concourse import mybir
    partition_name = (nc.partition_id_tensor.name
                      if nc.partition_id_tensor else None)
    for alloc in nc.m.functions[0].allocations:
        if not isinstance(alloc, mybir.MemoryLocationSet):
            continue
        name = alloc.memorylocations[0].name
        if alloc.kind == "ExternalInput":
            if name != partition_name:
                in_names.append(name)
        elif alloc.kind == "ExternalOutput":
            out_names.append(name)
            shape = tuple(alloc.tensor_shape)
            dtype = mybir.dt.np(alloc.dtype)
            out_avals.append(jax.core.ShapedArray(shape, dtype))
            zero_outs.append(np.zeros(shape, dtype))
    n_params = len(in_names)
    all_in_names = in_names + out_names + (
        [partition_name] if partition_name else [])

    def _body(*args):
        operands = list(args)
        if partition_name is not None:
            operands.append(bass2jax.partition_id_tensor())
        outs = bass2jax._bass_exec_p.bind(
            *operands,
            out_avals=tuple(out_avals),
            in_names=tuple(all_in_names),
            out_names=tuple(out_names),
            lowering_input_output_aliases=(),
            sim_require_finite=True,
            sim_require_nnan=True,
            nc=nc,
        )
        return tuple(outs)

    devices = jax.devices()[:NCORES]
    mesh = Mesh(np.asarray(devices), ("core",))
    n_outs = len(out_names)
    sharded = jax.jit(
        shard_map(_body, mesh=mesh,
                  in_specs=(PartitionSpec("core"),) * (n_params + n_outs),
                  out_specs=(PartitionSpec("core"),) * n_outs,
                  check_rep=False),
        donate_argnums=tuple(range(n_params, n_params + n_outs)),
        keep_unused=True)

    def runner(in_maps):
        concat_in = [np.concatenate([m[name] for m in in_maps], axis=0)
                     for name in in_names]
        concat_zeros = [np.zeros((NCORES * z.shape[0], *z.shape[1:]), z.dtype)
                        for z in zero_outs]
        out_arrs = sharded(*concat_in, *concat_zeros)
        return [
            {name: np.asarray(out_arrs[i]).reshape(NCORES, *out_avals[i].shape)[c]
             for i, name in enumerate(out_names)}
            for c in range(NCORES)]

    _CACHE["runner"] = runner
    return runner


def _refine(sim, dbn, qcols, db_classes, inv_qnorm):
    """Exact fp32 rescore of rows within REFINE_EPS of each query's device max
    (device sims are bf16-rounded; top1-top2 gaps are ~100x the bf16 error)."""
    mx = sim.max(axis=0)
    unit = np.empty(B2, dtype=np.float32)
    top = np.empty(B2, dtype=np.int32)
    cls = np.asarray(db_classes).astype(np.int32)
    for q in range(B2):
        cand = np.nonzero(sim[:, q] >= mx[q] - REFINE_EPS)[0]
        exact = (dbn[cand] @ qcols[:, q * NS:(q + 1) * NS]).max(axis=1)
        best = int(np.argmax(exact))
        unit[q] = exact[best] * inv_qnorm[q]
        top[q] = cls[cand[best]]
    return unit, top


def kernel(bert_input_ids, bert_offsets, database, db_classes,
           slidingWindowIndex, _trace=False):
    qt, inv_qnorm, qcols, ch15_zero = _prep_queries(
        bert_input_ids, bert_offsets, slidingWindowIndex)
    hi, lo, dbn = _prep_db(database)
    if not ch15_zero:
        # never per input spec (ids < 30522 < 2^15); exact host fallback
        sim = (dbn @ qcols).reshape(N_DB, B2, NS).max(axis=2)
        avg_sim = (sim.mean(axis=0) * inv_qnorm).astype(np.float32)
        unit_sim = (sim.max(axis=0) * inv_qnorm).astype(np.float32)
        top_cls = np.asarray(db_classes).astype(np.int32)[np.argmax(sim, axis=0)]
        if _trace:
            return (unit_sim, top_cls, avg_sim), None
        return unit_sim, top_cls, avg_sim
    in_maps = []
    for c in range(NCORES):
        m = {"db_hi": hi[c], "qt": qt}
        if USE_LO:
            m["db_lo"] = lo[c]
        in_maps.append(m)
    if _trace:
        res = _run_device(in_maps, trace=True,
                          trace_cores=list(range(NCORES)))
        results = res.results
    else:
        res = None
        results = _get_runner()(in_maps)
    sim = np.concatenate(
        [results[c]["out"].reshape(128, MT, B2).transpose(1, 0, 2)
         .reshape(NSH, B2) for c in range(NCORES)], axis=0)
    avg_sim = (sim.mean(axis=0) * inv_qnorm).astype(np.float32)
    if USE_LO:
        unit_sim = (sim.max(axis=0) * inv_qnorm).astype(np.float32)
        top_cls = np.asarray(db_classes).astype(np.int32)[
            np.argmax(sim, axis=0)]
    else:
        unit_sim, top_cls = _refine(sim, dbn, qcols, db_classes, inv_qnorm)
    if _trace:
        return (unit_sim, top_cls, avg_sim), res
    return unit_sim, top_cls, avg_sim

